# revision 15
# baseline (speedup 1.0000x reference)
"""Single-head masked attention (B=4, S=2048, D=1024, fp32) on 8 TRN2 NeuronCores.

Sharding: core c handles batch b=c//2, query half h=c%2 (1024 queries), with
K/V work over all 2048 keys of its batch. For h=1 cores the key axis is
rotated by 1024 on the host so every core runs the identical SPMD program
(attention is invariant to key permutation when the mask is permuted too).

The kernel exploits two algebraic reassociations that cut the matmul work
from 1280 to 1024 tile-matmuls per core:

1) scores^T = K @ Q^T = (x @ Wk^T + bk) @ Q^T
            = x @ (Wk^T @ Q^T)  [+ bk . Q^T, constant per query]
   The bias term is constant across keys for each query, so softmax's shift
   invariance cancels it EXACTLY -- bk is simply dropped. Computing
   G[d,q] = Wk^T @ Q^T first (2.1 GF) and then S^T = x @ G (4.3 GF) replaces
   K-projection (4.3) + scores (4.3). Bonus: G's lhsT is Wk in its NATIVE
   [e,d] layout, and K^T (8MB) is never materialized.

2) out = attnU @ (x @ Wv^T) / sumexp + bv
       = (attnU @ x) @ Wv^T / sumexp + bv
   Z^T[d,q] = x^T-weighted attention (4.3 GF) then out = Z^T.T @ Wv^T
   (2.1 GF) replaces V-projection (4.3) + PV (4.3). The value bias bv
   contributes exactly bv per row (softmax weights sum to 1) and is added in
   the final normalize op. V is never materialized (no DRAM spill).

Matmul layouts (contraction always on the partition dim, zero on-chip
transposes; host supplies xT=[d,s], xN=[s,d], wqT/wvT transposed, wkN native):
  Q^T[e,q]  : lhsT=WqT [d,e-col-tiles], rhs=xT [d,q]      (+bq per-partition)
  G[d,q]    : lhsT=WkN [e,d-slices],    rhs=Q^T [e,q]
  S^T[k,q]  : lhsT=xT  [d,k-slices],    rhs=G   [d,q]
  attnU^T   = exp(S^T/32 + mask_bias[k])  -- ONE fused ScalarE op per tile
              (masked lanes get -30000 -> exp underflows to exact 0; no
              max-subtraction needed: |s/32| <~ 6)
  sumexp[q] : lhsT=attnU^T [k,q-slices], rhs=ones [k,2]   (fp32r needs even N)
  Z^T[d,q]  : lhsT=xN [k,d-slices],      rhs=attnU^T [k,q]
  out[q,dv] : lhsT=Z^T [d,q-slices],     rhs=WvT [d,dv]
  final     : out = psum * (1/sumexp)[q] + bv_bcast  -- one DVE op

All matmuls run in float32r (fp32 bits at bf16-rate: 1 cycle/row for moving
free dim >= 256 vs 4 cycles/row for plain fp32; ~1.6e-4 component error;
HW-verified to accept raw fp32 bit patterns from DRAM directly).

Queue discipline (HWDGE issue is in-order per engine; a compute op waiting on
a semaphore would block DMA issues queued behind it): sync carries W loads +
xN streams + outputs; scalar carries x^T loads + constants (its only compute
is the phase-2 exps); vector does all PSUM->SBUF movement.
"""

from contextlib import ExitStack

import numpy as np

import concourse.bacc as bacc
import concourse.mybir as mybir
import concourse.tile as tile
from concourse.bass_utils import run_bass_kernel_spmd

D = 1024       # model dim = head dim
S = 2048       # sequence length (keys per core)
QL = 1024      # queries per core
N_CORES = 8
SCALE = 1.0 / 32.0   # 1/sqrt(D)
MASK_NEG = -30000.0

F32 = mybir.dt.float32
F32R = mybir.dt.float32r
AF = mybir.ActivationFunctionType
ALU = mybir.AluOpType


def _build_nc():
    nc = bacc.Bacc(None)

    xT = nc.declare_dram_parameter("xT", [D, S], F32R, isOutput=False)[:]
    xN = nc.declare_dram_parameter("xN", [S, D], F32R, isOutput=False)[:]
    wqT = nc.declare_dram_parameter("wqT", [D, D], F32R, isOutput=False)[:]
    wkN = nc.declare_dram_parameter("wkN", [D, D], F32R, isOutput=False)[:]
    wvT = nc.declare_dram_parameter("wvT", [D, D], F32R, isOutput=False)[:]
    bqT = nc.declare_dram_parameter("bqT", [128, 8], F32, isOutput=False)[:]
    mbT = nc.declare_dram_parameter("mbT", [128, 16], F32, isOutput=False)[:]
    bvb = nc.declare_dram_parameter("bvb", [128, D], F32, isOutput=False)[:]
    onesd = nc.declare_dram_parameter("onesd", [128, 2], F32R, isOutput=False)[:]
    out_d = nc.declare_dram_parameter("out", [QL, D], F32, isOutput=True)[:]

    with tile.TileContext(nc) as tc:
        _emit(nc, tc, xT, xN, wqT, wkN, wvT, bqT, mbT, bvb, onesd, out_d)
    nc.finalize()
    return nc


def _emit(nc, tc, xT, xN, wqT, wkN, wvT, bqT, mbT, bvb, onesd, out_d):
    with ExitStack() as ctx:
        consts = ctx.enter_context(tc.tile_pool(name="consts", bufs=1))

        # G[d,q] = Wk^T @ Q^T lives across both phases, 8 d-partition tiles.
        gpool = ctx.enter_context(tc.tile_pool(name="g", bufs=8))
        gt = [gpool.tile([128, QL], F32R, tag="gt", name=f"gt{m}")
              for m in range(8)]
        # xs tiles (S^T lhsT) live in an outer pool so their loads are not
        # gated on the phase-1 pool release -- they stream during G.
        xspool = ctx.enter_context(tc.tile_pool(name="xs", bufs=4))

        # ---------------- Phase 1: Q^T then G ----------------
        with (
            tc.tile_pool(name="proj", bufs=1) as pp,
            tc.tile_pool(name="projps", bufs=6, space="PSUM") as pps,
        ):
            # Q^T [e,q] as 8 e-partition tiles (phase-1 only).
            qt = [pp.tile([128, QL], F32R, tag="qt", bufs=8, name=f"qt{m}")
                  for m in range(8)]

            # wq split by e-column group so the first matmul group only waits
            # on its own 0.5 MB slice.
            wq = []
            for m in range(8):
                w = pp.tile([128, 8, 128], F32R, tag="w", bufs=16,
                            name=f"wq{m}")
                nc.sync.dma_start(
                    out=w,
                    in_=wqT[:, m * 128:(m + 1) * 128]
                    .rearrange("(a p) e -> p a e", p=128))
                wq.append(w)
            xq = []
            for c in range(2):
                x = pp.tile([128, 8, 512], F32R, tag="x", bufs=2, name=f"xq{c}")
                nc.scalar.dma_start(
                    out=x,
                    in_=xT[:, c * 512:(c + 1) * 512]
                    .rearrange("(a p) s -> p a s", p=128))
                xq.append(x)
            bq_sb = consts.tile([128, 8], F32, tag="bq", name="bq_sb")
            nc.scalar.dma_start(out=bq_sb, in_=bqT)
            mb_sb = consts.tile([128, 16], F32, tag="mb", name="mb_sb")
            nc.scalar.dma_start(out=mb_sb, in_=mbT)
            ones_sb = consts.tile([128, 2], F32R, tag="ones", name="ones_sb")
            nc.scalar.dma_start(out=ones_sb, in_=onesd)
            # Preload the exp table set while the PE is in the projections.
            warm = consts.tile([128, 2], F32, tag="warm", name="warm")
            nc.scalar.activation(warm, ones_sb, AF.Exp)

            # ---- Q^T = WqT.T @ xT[:, 0:1024]  (+ bq per-partition) ----
            for qc in range(2):
                for m in range(8):
                    ps = pps.tile([128, 512], F32, tag="ps", name=f"psq{qc}_{m}")
                    for dk in range(8):
                        nc.tensor.matmul(
                            ps, wq[m][:, dk, :], xq[qc][:, dk, :],
                            start=(dk == 0), stop=(dk == 7))
                    nc.vector.tensor_scalar_add(
                        qt[m][:, qc * 512:(qc + 1) * 512], ps, bq_sb[:, m:m + 1])

            # ---- G[d,q] = WkN.T @ Q^T  (Wk in native [e,d] layout) ----
            # wk tiles are [128e, 1024d] native rows: 4 KB/partition, same
            # slot size as the wq tiles, so they recycle the "w" tag slots.
            wk = []
            for ec in range(8):
                w = pp.tile([128, D], F32R, tag="w", bufs=16, name=f"wk{ec}")
                nc.sync.dma_start(out=w, in_=wkN[ec * 128:(ec + 1) * 128, :])
                wk.append(w)
            for dt in range(8):
                for qch in range(2):
                    ps = pps.tile([128, 512], F32, tag="ps",
                                  name=f"psg{dt}_{qch}")
                    for ec in range(8):
                        nc.tensor.matmul(
                            ps,
                            wk[ec][:, dt * 128:(dt + 1) * 128],
                            qt[ec][:, qch * 512:(qch + 1) * 512],
                            start=(ec == 0), stop=(ec == 7))
                    nc.vector.tensor_copy(
                        gt[dt][:, qch * 512:(qch + 1) * 512], ps)

        # ---------------- Phase 2: attention ----------------
        with (
            tc.tile_pool(name="att", bufs=1) as at_p,
            tc.tile_pool(name="aps1", bufs=2, space="PSUM") as aps,
            tc.tile_pool(name="aps2", bufs=4, space="PSUM") as zps,
        ):
            bvb_sb = at_p.tile([128, D], F32, tag="bvb", bufs=1, name="bvb_sb")
            nc.scalar.dma_start(out=bvb_sb, in_=bvb)
            # wv (= Wv^T rows, d-split) resident for the final out-matmul.
            wv = []
            for dt in range(8):
                w = at_p.tile([128, D], F32R, tag="wv", bufs=8, name=f"wv{dt}")
                nc.sync.dma_start(out=w, in_=wvT[dt * 128:(dt + 1) * 128, :])
                wv.append(w)

            # ---- S^T[k,q] = xT.T @ G -> fused mask+exp, both q-chunks ----
            at = [[], []]
            for kt_i in range(16):
                xs = xspool.tile([128, 8, 128], F32R, tag="xs",
                                 name=f"xs{kt_i}")
                nc.scalar.dma_start(
                    out=xs,
                    in_=xT[:, kt_i * 128:(kt_i + 1) * 128]
                    .rearrange("(a p) s -> p a s", p=128))
                for qc in range(2):
                    ps = aps.tile([128, 512], F32, tag="ps_s", bufs=2,
                                  name=f"pss{qc}_{kt_i}")
                    for dc in range(8):
                        nc.tensor.matmul(
                            ps,
                            xs[:, dc, :],
                            gt[dc][:, qc * 512:(qc + 1) * 512],
                            start=(dc == 0), stop=(dc == 7))
                    a = at_p.tile([128, 512], F32R, tag="at", bufs=32,
                                  name=f"at{qc}_{kt_i}")
                    nc.scalar.activation(
                        a, ps, AF.Exp,
                        bias=mb_sb[:, kt_i:kt_i + 1], scale=SCALE)
                    at[qc].append(a)

            for qc in range(2):
                # sumexp over k (partition dim) via ones-matmul, then 1/x
                recips = []
                for qs in range(4):
                    pss = aps.tile([128, 2], F32, tag="ps_sum", bufs=2,
                                   name=f"pssum{qc}_{qs}")
                    for kt_i in range(16):
                        nc.tensor.matmul(
                            pss,
                            at[qc][kt_i][:, qs * 128:(qs + 1) * 128],
                            ones_sb,
                            start=(kt_i == 0), stop=(kt_i == 15))
                    r = at_p.tile([128, 1], F32, tag="recip", bufs=8,
                                  name=f"r{qc}_{qs}")
                    nc.vector.reciprocal(r, pss[:, 0:1])
                    recips.append(r)

                # ---- Z^T[d,q] = xN.T @ attnU^T (4 d-tiles per xN pass) ----
                zt = []
                for dth in range(2):
                    pzs = [zps.tile([128, 512], F32, tag="ps_z",
                                    name=f"psz{qc}_{dth}_{j}")
                           for j in range(4)]
                    for kt_i in range(16):
                        xn = at_p.tile([128, 512], F32R, tag="xn", bufs=4,
                                       name=f"xn{qc}_{dth}_{kt_i}")
                        nc.sync.dma_start(
                            out=xn,
                            in_=xN[kt_i * 128:(kt_i + 1) * 128,
                                   dth * 512:(dth + 1) * 512])
                        for j in range(4):
                            nc.tensor.matmul(
                                pzs[j],
                                xn[:, j * 128:(j + 1) * 128],
                                at[qc][kt_i],
                                start=(kt_i == 0), stop=(kt_i == 15))
                    for j in range(4):
                        z = at_p.tile([128, 512], F32R, tag="zt", bufs=8,
                                      name=f"zt{qc}_{dth}_{j}")
                        nc.vector.tensor_copy(z, pzs[j])
                        zt.append(z)

                # ---- out[q,dv] = Z^T.T @ WvT * recip[q] + bv ----
                for qs in range(4):
                    for dvc in range(2):
                        ps = zps.tile([128, 512], F32, tag="ps_z",
                                      name=f"pso{qc}_{qs}_{dvc}")
                        for dt in range(8):
                            nc.tensor.matmul(
                                ps,
                                zt[dt][:, qs * 128:(qs + 1) * 128],
                                wv[dt][:, dvc * 512:(dvc + 1) * 512],
                                start=(dt == 0), stop=(dt == 7))
                        o = at_p.tile([128, 512], F32, tag="o", bufs=4,
                                      name=f"o{qc}_{qs}_{dvc}")
                        nc.vector.scalar_tensor_tensor(
                            o, ps, recips[qs], bvb_sb[:, dvc * 512:(dvc + 1) * 512],
                            op0=ALU.mult, op1=ALU.add)
                        row = (qc * 4 + qs) * 128
                        nc.sync.dma_start(
                            out=out_d[row:row + 128, dvc * 512:(dvc + 1) * 512],
                            in_=o)


def _prep_inputs(x, mask, Wq, bq, Wk, bk, Wv, bv):
    x = np.ascontiguousarray(np.asarray(x, dtype=np.float32))
    mask = np.asarray(mask, dtype=bool)
    Wq = np.asarray(Wq, dtype=np.float32)
    bq = np.asarray(bq, dtype=np.float32)
    Wk = np.ascontiguousarray(np.asarray(Wk, dtype=np.float32))
    Wv = np.asarray(Wv, dtype=np.float32)
    bv = np.asarray(bv, dtype=np.float32)
    del bk  # exactly cancelled by softmax shift invariance

    wqT = np.ascontiguousarray(Wq.T)
    wvT = np.ascontiguousarray(Wv.T)
    bqT = np.ascontiguousarray(bq.reshape(8, 128).T)
    bvb = np.ascontiguousarray(np.broadcast_to(bv, (128, D)))
    ones = np.ones((128, 2), dtype=np.float32)

    in_maps = []
    for c in range(N_CORES):
        b, h = divmod(c, 2)
        if h == 0:
            xN_c = x[b]
            mask_c = mask[b]
        else:
            xN_c = np.concatenate([x[b, QL:], x[b, :QL]], axis=0)
            mask_c = np.concatenate([mask[b, QL:], mask[b, :QL]])
        xN_c = np.ascontiguousarray(xN_c)
        xT_c = np.ascontiguousarray(xN_c.T)
        mb = np.where(mask_c, 0.0, MASK_NEG).astype(np.float32)
        mbT = np.ascontiguousarray(mb.reshape(16, 128).T)
        in_maps.append({
            "xT": xT_c, "xN": xN_c, "wqT": wqT, "wkN": Wk, "wvT": wvT,
            "bqT": bqT, "mbT": mbT, "bvb": bvb, "onesd": ones,
        })
    return in_maps


def run(x, mask, Wq, bq, Wk, bk, Wv, bv, trace=False):
    """Build + run; returns (output, BassKernelResults)."""
    in_maps = _prep_inputs(x, mask, Wq, bq, Wk, bk, Wv, bv)
    nc = _build_nc()
    res = run_bass_kernel_spmd(nc, in_maps, list(range(N_CORES)), trace=trace)
    out = np.empty((4, S, D), dtype=np.float32)
    for c in range(N_CORES):
        b, h = divmod(c, 2)
        out[b, h * QL:(h + 1) * QL, :] = res.results[c]["out"]
    return out, res


def kernel(x, mask, Wq, bq, Wk, bk, Wv, bv):
    out, _ = run(x, mask, Wq, bq, Wk, bk, Wv, bv)
    return out


# revision 17
# speedup vs baseline: 1.1107x; 1.1107x over previous
"""Single-head masked attention (B=4, S=2048, D=1024, fp32) on 8 TRN2 NeuronCores.

Sharding: core c handles batch b=c//2, query half h=c%2 (1024 queries), with
K/V work over all 2048 keys of its batch. For h=1 cores the key axis is
rotated by 1024 on the host so every core runs the identical SPMD program
(attention is invariant to key permutation when the mask is permuted too).

The kernel exploits two algebraic reassociations that cut the matmul work
from 1280 to 1024 tile-matmuls per core:

1) scores^T = K @ Q^T = (x @ Wk^T + bk) @ Q^T
            = x @ (Wk^T @ Q^T)  [+ bk . Q^T, constant per query]
   The bias term is constant across keys for each query, so softmax's shift
   invariance cancels it EXACTLY -- bk is simply dropped. Computing
   G[d,q] = Wk^T @ Q^T first (2.1 GF) and then S^T = x @ G (4.3 GF) replaces
   K-projection (4.3) + scores (4.3). Bonus: G's lhsT is Wk in its NATIVE
   [e,d] layout, and K^T (8MB) is never materialized.

2) out = attnU @ (x @ Wv^T) / sumexp + bv
       = (attnU @ x) @ Wv^T / sumexp + bv
   Z^T[d,q] = x^T-weighted attention (4.3 GF) then out = Z^T.T @ Wv^T
   (2.1 GF) replaces V-projection (4.3) + PV (4.3). The value bias bv
   contributes exactly bv per row (softmax weights sum to 1) and is added in
   the final normalize op. V is never materialized (no DRAM spill).

Matmul layouts (contraction always on the partition dim, zero on-chip
transposes; host supplies xT=[d,s], xN=[s,d], wqT/wvT transposed, wkN native):
  Q^T[e,q]  : lhsT=WqT [d,e-col-tiles], rhs=xT [d,q]      (+bq per-partition)
  G[d,q]    : lhsT=WkN [e,d-slices],    rhs=Q^T [e,q]
  S^T[k,q]  : lhsT=xT  [d,k-slices],    rhs=G   [d,q]
  attnU^T   = exp(S^T/32 + mask_bias[k])  -- ONE fused ScalarE op per tile
              (masked lanes get -30000 -> exp underflows to exact 0; no
              max-subtraction needed: |s/32| <~ 6)
  sumexp[q] : lhsT=attnU^T [k,q-slices], rhs=ones [k,2]   (fp32r needs even N)
  Z^T[d,q]  : lhsT=xN [k,d-slices],      rhs=attnU^T [k,q]
  out[q,dv] : lhsT=Z^T [d,q-slices],     rhs=WvT [d,dv]
  final     : out = psum * (1/sumexp)[q] + bv_bcast  -- one DVE op

All matmuls run in float32r (fp32 bits at bf16-rate: 1 cycle/row for moving
free dim >= 256 vs 4 cycles/row for plain fp32; ~1.6e-4 component error;
HW-verified to accept raw fp32 bit patterns from DRAM directly).

Queue discipline (HWDGE issue is in-order per engine; a compute op waiting on
a semaphore would block DMA issues queued behind it): sync carries W loads +
xN streams + outputs; scalar carries x^T loads + constants (its only compute
is the phase-2 exps); vector does all PSUM->SBUF movement.
"""

from contextlib import ExitStack

import numpy as np

import concourse.bacc as bacc
import concourse.mybir as mybir
import concourse.tile as tile
from concourse.bass_utils import run_bass_kernel_spmd

D = 1024       # model dim = head dim
S = 2048       # sequence length (keys per core)
QL = 1024      # queries per core
N_CORES = 8
SCALE = 1.0 / 32.0   # 1/sqrt(D)
MASK_NEG = -30000.0

F32 = mybir.dt.float32
F32R = mybir.dt.float32r
AF = mybir.ActivationFunctionType
ALU = mybir.AluOpType


def _build_nc():
    nc = bacc.Bacc(None)

    xT = nc.declare_dram_parameter("xT", [D, S], F32R, isOutput=False)[:]
    xN = nc.declare_dram_parameter("xN", [S, D], F32R, isOutput=False)[:]
    wqT = nc.declare_dram_parameter("wqT", [D, D], F32R, isOutput=False)[:]
    wkN = nc.declare_dram_parameter("wkN", [D, D], F32R, isOutput=False)[:]
    wvT = nc.declare_dram_parameter("wvT", [D, D], F32R, isOutput=False)[:]
    bqT = nc.declare_dram_parameter("bqT", [128, 8], F32, isOutput=False)[:]
    mbT = nc.declare_dram_parameter("mbT", [128, 16], F32, isOutput=False)[:]
    bvb = nc.declare_dram_parameter("bvb", [128, D], F32, isOutput=False)[:]
    onesd = nc.declare_dram_parameter("onesd", [128, 2], F32R, isOutput=False)[:]
    out_d = nc.declare_dram_parameter("out", [QL, D], F32, isOutput=True)[:]

    with tile.TileContext(nc) as tc:
        _emit(nc, tc, xT, xN, wqT, wkN, wvT, bqT, mbT, bvb, onesd, out_d)
    nc.finalize()
    return nc


def _emit(nc, tc, xT, xN, wqT, wkN, wvT, bqT, mbT, bvb, onesd, out_d):
    with ExitStack() as ctx:
        consts = ctx.enter_context(tc.tile_pool(name="consts", bufs=1))

        # G[d,q] = Wk^T @ Q^T lives across both phases, 8 d-partition tiles.
        gpool = ctx.enter_context(tc.tile_pool(name="g", bufs=8))
        gt = [gpool.tile([128, QL], F32R, tag="gt", name=f"gt{m}")
              for m in range(8)]
        # xs tiles (S^T lhsT) live in an outer pool so their loads are not
        # gated on the phase-1 pool release -- they stream during G.
        xspool = ctx.enter_context(tc.tile_pool(name="xs", bufs=4))
        # One PSUM pool for the whole kernel: no pool-release barrier at the
        # phase transition. "ps" (6 banks) serves projections, scores, Z and
        # out; "ps_sum" (2 banks) serves the sumexp accumulators.
        pps = ctx.enter_context(tc.tile_pool(name="ps", bufs=6, space="PSUM"))

        # ---------------- Phase 1: Q^T then G ----------------
        with tc.tile_pool(name="proj", bufs=1) as pp:
            # Q^T [e,q] as 8 e-partition tiles (phase-1 only).
            qt = [pp.tile([128, QL], F32R, tag="qt", bufs=8, name=f"qt{m}")
                  for m in range(8)]

            # wq split by e-column group so the first matmul group only waits
            # on its own 0.5 MB slice.
            wq = []
            for m in range(8):
                w = pp.tile([128, 8, 128], F32R, tag="w", bufs=16,
                            name=f"wq{m}")
                nc.sync.dma_start(
                    out=w,
                    in_=wqT[:, m * 128:(m + 1) * 128]
                    .rearrange("(a p) e -> p a e", p=128))
                wq.append(w)
            xq = []
            for c in range(2):
                x = pp.tile([128, 8, 512], F32R, tag="x", bufs=2, name=f"xq{c}")
                nc.scalar.dma_start(
                    out=x,
                    in_=xT[:, c * 512:(c + 1) * 512]
                    .rearrange("(a p) s -> p a s", p=128))
                xq.append(x)
            bq_sb = consts.tile([128, 8], F32, tag="bq", name="bq_sb")
            nc.scalar.dma_start(out=bq_sb, in_=bqT)
            mb_sb = consts.tile([128, 16], F32, tag="mb", name="mb_sb")
            nc.scalar.dma_start(out=mb_sb, in_=mbT)
            ones_sb = consts.tile([128, 2], F32R, tag="ones", name="ones_sb")
            nc.scalar.dma_start(out=ones_sb, in_=onesd)
            # Preload the exp table set while the PE is in the projections.
            warm = consts.tile([128, 2], F32, tag="warm", name="warm")
            nc.scalar.activation(warm, ones_sb, AF.Exp)

            # ---- Q^T = WqT.T @ xT[:, 0:1024]  (+ bq per-partition) ----
            for qc in range(2):
                for m in range(8):
                    ps = pps.tile([128, 512], F32, tag="ps", name=f"psq{qc}_{m}")
                    for dk in range(8):
                        nc.tensor.matmul(
                            ps, wq[m][:, dk, :], xq[qc][:, dk, :],
                            start=(dk == 0), stop=(dk == 7))
                    nc.vector.tensor_scalar_add(
                        qt[m][:, qc * 512:(qc + 1) * 512], ps, bq_sb[:, m:m + 1])

            # ---- G[d,q] = WkN.T @ Q^T  (Wk in native [e,d] layout) ----
            # wk tiles are [128e, 1024d] native rows: 4 KB/partition, same
            # slot size as the wq tiles, so they recycle the "w" tag slots.
            wk = []
            for ec in range(8):
                w = pp.tile([128, D], F32R, tag="w", bufs=16, name=f"wk{ec}")
                nc.sync.dma_start(out=w, in_=wkN[ec * 128:(ec + 1) * 128, :])
                wk.append(w)
            for dt in range(8):
                for qch in range(2):
                    ps = pps.tile([128, 512], F32, tag="ps",
                                  name=f"psg{dt}_{qch}")
                    for ec in range(8):
                        nc.tensor.matmul(
                            ps,
                            wk[ec][:, dt * 128:(dt + 1) * 128],
                            qt[ec][:, qch * 512:(qch + 1) * 512],
                            start=(ec == 0), stop=(ec == 7))
                    nc.vector.tensor_copy(
                        gt[dt][:, qch * 512:(qch + 1) * 512], ps)

        # ---------------- Phase 2: attention ----------------
        with tc.tile_pool(name="att", bufs=1) as at_p:
            bvb_sb = at_p.tile([128, D], F32, tag="bvb", bufs=1, name="bvb_sb")
            nc.scalar.dma_start(out=bvb_sb, in_=bvb)
            # wv (= Wv^T rows, d-split) resident for the final out-matmul.
            wv = []
            for dt in range(8):
                w = at_p.tile([128, D], F32R, tag="wv", bufs=8, name=f"wv{dt}")
                nc.sync.dma_start(out=w, in_=wvT[dt * 128:(dt + 1) * 128, :])
                wv.append(w)

            # ---- S^T[k,q] = xT.T @ G -> fused mask+exp, both q-chunks ----
            at = [[], []]
            for kt_i in range(16):
                xs = xspool.tile([128, 8, 128], F32R, tag="xs",
                                 name=f"xs{kt_i}")
                nc.scalar.dma_start(
                    out=xs,
                    in_=xT[:, kt_i * 128:(kt_i + 1) * 128]
                    .rearrange("(a p) s -> p a s", p=128))
                for qc in range(2):
                    ps = pps.tile([128, 512], F32, tag="ps", name=f"pss{qc}_{kt_i}")
                    for dc in range(8):
                        nc.tensor.matmul(
                            ps,
                            xs[:, dc, :],
                            gt[dc][:, qc * 512:(qc + 1) * 512],
                            start=(dc == 0), stop=(dc == 7))
                    a = at_p.tile([128, 512], F32R, tag="at", bufs=32,
                                  name=f"at{qc}_{kt_i}")
                    nc.scalar.activation(
                        a, ps, AF.Exp,
                        bias=mb_sb[:, kt_i:kt_i + 1], scale=SCALE)
                    at[qc].append(a)

            for qc in range(2):
                # ---- Z^T[d,q] = xN.T @ attnU^T (4 d-tiles per xN pass) ----
                zt = []
                for dth in range(2):
                    pzs = [pps.tile([128, 512], F32, tag="ps", name=f"psz{qc}_{dth}_{j}")
                           for j in range(4)]
                    for kt_i in range(16):
                        xn = at_p.tile([128, 512], F32R, tag="xn", bufs=8,
                                       name=f"xn{qc}_{dth}_{kt_i}")
                        nc.sync.dma_start(
                            out=xn,
                            in_=xN[kt_i * 128:(kt_i + 1) * 128,
                                   dth * 512:(dth + 1) * 512])
                        for j in range(4):
                            nc.tensor.matmul(
                                pzs[j],
                                xn[:, j * 128:(j + 1) * 128],
                                at[qc][kt_i],
                                start=(kt_i == 0), stop=(kt_i == 15))
                    for j in range(4):
                        z = at_p.tile([128, 512], F32R, tag="zt", bufs=8,
                                      name=f"zt{qc}_{dth}_{j}")
                        nc.vector.tensor_copy(z, pzs[j])
                        zt.append(z)

                # sumexp over k (partition dim) via ones-matmul, then 1/x
                recips = []
                for qs in range(4):
                    pss = pps.tile([128, 2], F32, tag="ps_sum", bufs=2,
                                   name=f"pssum{qc}_{qs}")
                    for kt_i in range(16):
                        nc.tensor.matmul(
                            pss,
                            at[qc][kt_i][:, qs * 128:(qs + 1) * 128],
                            ones_sb,
                            start=(kt_i == 0), stop=(kt_i == 15))
                    r = at_p.tile([128, 1], F32, tag="recip", bufs=8,
                                  name=f"r{qc}_{qs}")
                    nc.vector.reciprocal(r, pss[:, 0:1])
                    recips.append(r)

                # ---- out[q,dv] = Z^T.T @ WvT * recip[q] + bv ----
                for qs in range(4):
                    for dvc in range(2):
                        ps = pps.tile([128, 512], F32, tag="ps", name=f"pso{qc}_{qs}_{dvc}")
                        for dt in range(8):
                            nc.tensor.matmul(
                                ps,
                                zt[dt][:, qs * 128:(qs + 1) * 128],
                                wv[dt][:, dvc * 512:(dvc + 1) * 512],
                                start=(dt == 0), stop=(dt == 7))
                        o = at_p.tile([128, 512], F32, tag="o", bufs=4,
                                      name=f"o{qc}_{qs}_{dvc}")
                        nc.vector.scalar_tensor_tensor(
                            o, ps, recips[qs], bvb_sb[:, dvc * 512:(dvc + 1) * 512],
                            op0=ALU.mult, op1=ALU.add)
                        row = (qc * 4 + qs) * 128
                        nc.sync.dma_start(
                            out=out_d[row:row + 128, dvc * 512:(dvc + 1) * 512],
                            in_=o)


def _prep_inputs(x, mask, Wq, bq, Wk, bk, Wv, bv):
    x = np.ascontiguousarray(np.asarray(x, dtype=np.float32))
    mask = np.asarray(mask, dtype=bool)
    Wq = np.asarray(Wq, dtype=np.float32)
    bq = np.asarray(bq, dtype=np.float32)
    Wk = np.ascontiguousarray(np.asarray(Wk, dtype=np.float32))
    Wv = np.asarray(Wv, dtype=np.float32)
    bv = np.asarray(bv, dtype=np.float32)
    del bk  # exactly cancelled by softmax shift invariance

    wqT = np.ascontiguousarray(Wq.T)
    wvT = np.ascontiguousarray(Wv.T)
    bqT = np.ascontiguousarray(bq.reshape(8, 128).T)
    bvb = np.ascontiguousarray(np.broadcast_to(bv, (128, D)))
    ones = np.ones((128, 2), dtype=np.float32)

    in_maps = []
    for c in range(N_CORES):
        b, h = divmod(c, 2)
        if h == 0:
            xN_c = x[b]
            mask_c = mask[b]
        else:
            xN_c = np.concatenate([x[b, QL:], x[b, :QL]], axis=0)
            mask_c = np.concatenate([mask[b, QL:], mask[b, :QL]])
        xN_c = np.ascontiguousarray(xN_c)
        xT_c = np.ascontiguousarray(xN_c.T)
        mb = np.where(mask_c, 0.0, MASK_NEG).astype(np.float32)
        mbT = np.ascontiguousarray(mb.reshape(16, 128).T)
        in_maps.append({
            "xT": xT_c, "xN": xN_c, "wqT": wqT, "wkN": Wk, "wvT": wvT,
            "bqT": bqT, "mbT": mbT, "bvb": bvb, "onesd": ones,
        })
    return in_maps


def run(x, mask, Wq, bq, Wk, bk, Wv, bv, trace=False):
    """Build + run; returns (output, BassKernelResults)."""
    in_maps = _prep_inputs(x, mask, Wq, bq, Wk, bk, Wv, bv)
    nc = _build_nc()
    res = run_bass_kernel_spmd(nc, in_maps, list(range(N_CORES)), trace=trace)
    out = np.empty((4, S, D), dtype=np.float32)
    for c in range(N_CORES):
        b, h = divmod(c, 2)
        out[b, h * QL:(h + 1) * QL, :] = res.results[c]["out"]
    return out, res


def kernel(x, mask, Wq, bq, Wk, bk, Wv, bv):
    out, _ = run(x, mask, Wq, bq, Wk, bk, Wv, bv)
    return out


# revision 18
# speedup vs baseline: 1.1748x; 1.0577x over previous
"""Single-head masked attention (B=4, S=2048, D=1024, fp32) on 8 TRN2 NeuronCores.

Sharding: core c handles batch b=c//2, query half h=c%2 (1024 queries), with
K/V work over all 2048 keys of its batch. For h=1 cores the key axis is
rotated by 1024 on the host so every core runs the identical SPMD program
(attention is invariant to key permutation when the mask is permuted too).

The kernel exploits two algebraic reassociations that cut the matmul work
from 1280 to 1024 tile-matmuls per core:

1) scores^T = K @ Q^T = (x @ Wk^T + bk) @ Q^T
            = x @ (Wk^T @ Q^T)  [+ bk . Q^T, constant per query]
   The bias term is constant across keys for each query, so softmax's shift
   invariance cancels it EXACTLY -- bk is simply dropped. Computing
   G[d,q] = Wk^T @ Q^T first (2.1 GF) and then S^T = x @ G (4.3 GF) replaces
   K-projection (4.3) + scores (4.3). Bonus: G's lhsT is Wk in its NATIVE
   [e,d] layout, and K^T (8MB) is never materialized.

2) out = attnU @ (x @ Wv^T) / sumexp + bv
       = (attnU @ x) @ Wv^T / sumexp + bv
   Z^T[d,q] = x^T-weighted attention (4.3 GF) then out = Z^T.T @ Wv^T
   (2.1 GF) replaces V-projection (4.3) + PV (4.3). The value bias bv
   contributes exactly bv per row (softmax weights sum to 1) and is added in
   the final normalize op. V is never materialized (no DRAM spill).

Matmul layouts (contraction always on the partition dim, zero on-chip
transposes; host supplies xT=[d,s], xN=[s,d], wqT/wvT transposed, wkN native):
  Q^T[e,q]  : lhsT=WqT [d,e-col-tiles], rhs=xT [d,q]      (+bq per-partition)
  G[d,q]    : lhsT=WkN [e,d-slices],    rhs=Q^T [e,q]
  S^T[k,q]  : lhsT=xT  [d,k-slices],    rhs=G   [d,q]
  attnU^T   = exp(S^T/32 + mask_bias[k])  -- ONE fused ScalarE op per tile
              (masked lanes get -30000 -> exp underflows to exact 0; no
              max-subtraction needed: |s/32| <~ 6)
  sumexp[q] : lhsT=attnU^T [k,q-slices], rhs=ones [k,2]   (fp32r needs even N)
  Z^T[d,q]  : lhsT=xN [k,d-slices],      rhs=attnU^T [k,q]
  out[q,dv] : lhsT=Z^T [d,q-slices],     rhs=WvT [d,dv]
  final     : out = psum * (1/sumexp)[q] + bv_bcast  -- one DVE op

All matmuls run in float32r (fp32 bits at bf16-rate: 1 cycle/row for moving
free dim >= 256 vs 4 cycles/row for plain fp32; ~1.6e-4 component error;
HW-verified to accept raw fp32 bit patterns from DRAM directly).

Queue discipline (HWDGE issue is in-order per engine; a compute op waiting on
a semaphore would block DMA issues queued behind it): sync carries W loads +
xN streams + outputs; scalar carries x^T loads + constants (its only compute
is the phase-2 exps); vector does all PSUM->SBUF movement.
"""

from contextlib import ExitStack

import numpy as np

import concourse.bacc as bacc
import concourse.mybir as mybir
import concourse.tile as tile
from concourse.bass_utils import run_bass_kernel_spmd

D = 1024       # model dim = head dim
S = 2048       # sequence length (keys per core)
QL = 1024      # queries per core
N_CORES = 8
SCALE = 1.0 / 32.0   # 1/sqrt(D)
MASK_NEG = -30000.0

F32 = mybir.dt.float32
F32R = mybir.dt.float32r
AF = mybir.ActivationFunctionType
ALU = mybir.AluOpType


def _build_nc():
    nc = bacc.Bacc(None)

    xT = nc.declare_dram_parameter("xT", [D, S], F32R, isOutput=False)[:]
    xN = nc.declare_dram_parameter("xN", [S, D], F32R, isOutput=False)[:]
    wqT = nc.declare_dram_parameter("wqT", [D, D], F32R, isOutput=False)[:]
    wkN = nc.declare_dram_parameter("wkN", [D, D], F32R, isOutput=False)[:]
    wvT = nc.declare_dram_parameter("wvT", [D, D], F32R, isOutput=False)[:]
    bqT = nc.declare_dram_parameter("bqT", [128, 8], F32, isOutput=False)[:]
    mbT = nc.declare_dram_parameter("mbT", [128, 16], F32, isOutput=False)[:]
    bvb = nc.declare_dram_parameter("bvb", [128, D], F32, isOutput=False)[:]
    onesd = nc.declare_dram_parameter("onesd", [128, 2], F32R, isOutput=False)[:]
    out_d = nc.declare_dram_parameter("out", [QL, D], F32, isOutput=True)[:]

    with tile.TileContext(nc) as tc:
        _emit(nc, tc, xT, xN, wqT, wkN, wvT, bqT, mbT, bvb, onesd, out_d)
    nc.finalize()
    return nc


def _emit(nc, tc, xT, xN, wqT, wkN, wvT, bqT, mbT, bvb, onesd, out_d):
    with ExitStack() as ctx:
        consts = ctx.enter_context(tc.tile_pool(name="consts", bufs=1))

        # G[d,q] = Wk^T @ Q^T lives across both phases, 8 d-partition tiles.
        gpool = ctx.enter_context(tc.tile_pool(name="g", bufs=8))
        gt = [gpool.tile([128, QL], F32R, tag="gt", name=f"gt{m}")
              for m in range(8)]
        # xs tiles (S^T lhsT) live in an outer pool so their loads are not
        # gated on the phase-1 pool release -- they stream during G.
        xspool = ctx.enter_context(tc.tile_pool(name="xs", bufs=4))
        # One PSUM pool for the whole kernel: no pool-release barrier at the
        # phase transition. "ps" (6 banks) serves projections, scores, Z and
        # out; "ps_sum" (2 banks) serves the sumexp accumulators.
        pps = ctx.enter_context(tc.tile_pool(name="ps", bufs=6, space="PSUM"))

        # ---------------- Phase 1: Q^T then G ----------------
        with tc.tile_pool(name="proj", bufs=1) as pp:
            # Q^T [e,q] as 8 e-partition tiles (phase-1 only).
            qt = [pp.tile([128, QL], F32R, tag="qt", bufs=8, name=f"qt{m}")
                  for m in range(8)]

            # wq split by e-column group so the first matmul group only waits
            # on its own 0.5 MB slice.
            wq = []
            for m in range(8):
                w = pp.tile([128, 8, 128], F32R, tag="w", bufs=16,
                            name=f"wq{m}")
                nc.sync.dma_start(
                    out=w,
                    in_=wqT[:, m * 128:(m + 1) * 128]
                    .rearrange("(a p) e -> p a e", p=128))
                wq.append(w)
            xq = []
            for c in range(2):
                x = pp.tile([128, 8, 512], F32R, tag="x", bufs=2, name=f"xq{c}")
                nc.scalar.dma_start(
                    out=x,
                    in_=xT[:, c * 512:(c + 1) * 512]
                    .rearrange("(a p) s -> p a s", p=128))
                xq.append(x)
            bq_sb = consts.tile([128, 8], F32, tag="bq", name="bq_sb")
            nc.scalar.dma_start(out=bq_sb, in_=bqT)
            mb_sb = consts.tile([128, 16], F32, tag="mb", name="mb_sb")
            nc.scalar.dma_start(out=mb_sb, in_=mbT)
            ones_sb = consts.tile([128, 2], F32R, tag="ones", name="ones_sb")
            nc.scalar.dma_start(out=ones_sb, in_=onesd)
            # Preload the exp table set while the PE is in the projections.
            warm = consts.tile([128, 2], F32, tag="warm", name="warm")
            nc.scalar.activation(warm, ones_sb, AF.Exp)

            # ---- Q^T = WqT.T @ xT[:, 0:1024]  (+ bq per-partition) ----
            for qc in range(2):
                for m in range(8):
                    ps = pps.tile([128, 512], F32, tag="ps", name=f"psq{qc}_{m}")
                    for dk in range(8):
                        nc.tensor.matmul(
                            ps, wq[m][:, dk, :], xq[qc][:, dk, :],
                            start=(dk == 0), stop=(dk == 7))
                    nc.vector.tensor_scalar_add(
                        qt[m][:, qc * 512:(qc + 1) * 512], ps, bq_sb[:, m:m + 1])

            # ---- G[d,q] = WkN.T @ Q^T  (Wk in native [e,d] layout) ----
            # wk tiles are [128e, 1024d] native rows: 4 KB/partition, same
            # slot size as the wq tiles, so they recycle the "w" tag slots.
            wk = []
            for ec in range(8):
                w = pp.tile([128, D], F32R, tag="w", bufs=16, name=f"wk{ec}")
                nc.sync.dma_start(out=w, in_=wkN[ec * 128:(ec + 1) * 128, :])
                wk.append(w)
            for dt in range(8):
                for qch in range(2):
                    ps = pps.tile([128, 512], F32, tag="ps",
                                  name=f"psg{dt}_{qch}")
                    for ec in range(8):
                        nc.tensor.matmul(
                            ps,
                            wk[ec][:, dt * 128:(dt + 1) * 128],
                            qt[ec][:, qch * 512:(qch + 1) * 512],
                            start=(ec == 0), stop=(ec == 7))
                    nc.vector.tensor_copy(
                        gt[dt][:, qch * 512:(qch + 1) * 512], ps)

        # ---------------- Phase 2: attention ----------------
        with tc.tile_pool(name="att", bufs=1) as at_p:
            bvb_sb = at_p.tile([128, D], F32, tag="bvb", bufs=1, name="bvb_sb")
            nc.scalar.dma_start(out=bvb_sb, in_=bvb)
            # wv (= Wv^T rows, d-split) resident for the final out-matmul.
            wv = []
            for dt in range(8):
                w = at_p.tile([128, D], F32R, tag="wv", bufs=8, name=f"wv{dt}")
                nc.sync.dma_start(out=w, in_=wvT[dt * 128:(dt + 1) * 128, :])
                wv.append(w)

            # ---- S^T[k,q] = xT.T @ G -> fused mask+exp, both q-chunks ----
            at = [[], []]
            for kt_i in range(16):
                xs = xspool.tile([128, 8, 128], F32R, tag="xs",
                                 name=f"xs{kt_i}")
                nc.scalar.dma_start(
                    out=xs,
                    in_=xT[:, kt_i * 128:(kt_i + 1) * 128]
                    .rearrange("(a p) s -> p a s", p=128))
                for qc in range(2):
                    ps = pps.tile([128, 512], F32, tag="ps", name=f"pss{qc}_{kt_i}")
                    for dc in range(8):
                        nc.tensor.matmul(
                            ps,
                            xs[:, dc, :],
                            gt[dc][:, qc * 512:(qc + 1) * 512],
                            start=(dc == 0), stop=(dc == 7))
                    a = at_p.tile([128, 512], F32R, tag="at", bufs=32,
                                  name=f"at{qc}_{kt_i}")
                    nc.scalar.activation(
                        a, ps, AF.Exp,
                        bias=mb_sb[:, kt_i:kt_i + 1], scale=SCALE)
                    at[qc].append(a)

            for qc in range(2):
                # ---- sumexp as a [2,512] row: ones-lhsT matmul (M=2), then
                # reciprocal + GpSimd partition-broadcast; the normalize is
                # folded into the Z^T psum->SBUF copy as a tensor_mul. ----
                srow = pps.tile([2, 512], F32, tag="ps_sum", bufs=2,
                                name=f"srow{qc}")
                for kt_i in range(16):
                    nc.tensor.matmul(
                        srow, ones_sb, at[qc][kt_i],
                        start=(kt_i == 0), stop=(kt_i == 15))
                rrow = at_p.tile([2, 512], F32, tag="rrow", bufs=2,
                                 name=f"rrow{qc}")
                nc.vector.reciprocal(rrow, srow)
                rb = at_p.tile([128, 512], F32, tag="rb", bufs=2,
                               name=f"rb{qc}")
                nc.gpsimd.partition_broadcast(rb, rrow[0:1, :], channels=128)

                # ---- Z^T[d,q] = xN.T @ attnU^T (4 d-tiles per xN pass) ----
                zt = []
                for dth in range(2):
                    pzs = [pps.tile([128, 512], F32, tag="ps", name=f"psz{qc}_{dth}_{j}")
                           for j in range(4)]
                    for kt_i in range(16):
                        xn = at_p.tile([128, 512], F32R, tag="xn", bufs=8,
                                       name=f"xn{qc}_{dth}_{kt_i}")
                        nc.sync.dma_start(
                            out=xn,
                            in_=xN[kt_i * 128:(kt_i + 1) * 128,
                                   dth * 512:(dth + 1) * 512])
                        for j in range(4):
                            nc.tensor.matmul(
                                pzs[j],
                                xn[:, j * 128:(j + 1) * 128],
                                at[qc][kt_i],
                                start=(kt_i == 0), stop=(kt_i == 15))
                    for j in range(4):
                        z = at_p.tile([128, 512], F32R, tag="zt", bufs=8,
                                      name=f"zt{qc}_{dth}_{j}")
                        nc.vector.tensor_mul(z, pzs[j], rb)
                        zt.append(z)

                # ---- out[q,dv] = Z^T.T @ WvT * recip[q] + bv ----
                for qs in range(4):
                    for dvc in range(2):
                        ps = pps.tile([128, 512], F32, tag="ps", name=f"pso{qc}_{qs}_{dvc}")
                        for dt in range(8):
                            nc.tensor.matmul(
                                ps,
                                zt[dt][:, qs * 128:(qs + 1) * 128],
                                wv[dt][:, dvc * 512:(dvc + 1) * 512],
                                start=(dt == 0), stop=(dt == 7))
                        o = at_p.tile([128, 512], F32, tag="o", bufs=4,
                                      name=f"o{qc}_{qs}_{dvc}")
                        nc.vector.tensor_add(
                            o, ps, bvb_sb[:, dvc * 512:(dvc + 1) * 512])
                        row = (qc * 4 + qs) * 128
                        nc.sync.dma_start(
                            out=out_d[row:row + 128, dvc * 512:(dvc + 1) * 512],
                            in_=o)


def _prep_inputs(x, mask, Wq, bq, Wk, bk, Wv, bv):
    x = np.ascontiguousarray(np.asarray(x, dtype=np.float32))
    mask = np.asarray(mask, dtype=bool)
    Wq = np.asarray(Wq, dtype=np.float32)
    bq = np.asarray(bq, dtype=np.float32)
    Wk = np.ascontiguousarray(np.asarray(Wk, dtype=np.float32))
    Wv = np.asarray(Wv, dtype=np.float32)
    bv = np.asarray(bv, dtype=np.float32)
    del bk  # exactly cancelled by softmax shift invariance

    wqT = np.ascontiguousarray(Wq.T)
    wvT = np.ascontiguousarray(Wv.T)
    bqT = np.ascontiguousarray(bq.reshape(8, 128).T)
    bvb = np.ascontiguousarray(np.broadcast_to(bv, (128, D)))
    ones = np.ones((128, 2), dtype=np.float32)

    in_maps = []
    for c in range(N_CORES):
        b, h = divmod(c, 2)
        if h == 0:
            xN_c = x[b]
            mask_c = mask[b]
        else:
            xN_c = np.concatenate([x[b, QL:], x[b, :QL]], axis=0)
            mask_c = np.concatenate([mask[b, QL:], mask[b, :QL]])
        xN_c = np.ascontiguousarray(xN_c)
        xT_c = np.ascontiguousarray(xN_c.T)
        mb = np.where(mask_c, 0.0, MASK_NEG).astype(np.float32)
        mbT = np.ascontiguousarray(mb.reshape(16, 128).T)
        in_maps.append({
            "xT": xT_c, "xN": xN_c, "wqT": wqT, "wkN": Wk, "wvT": wvT,
            "bqT": bqT, "mbT": mbT, "bvb": bvb, "onesd": ones,
        })
    return in_maps


def run(x, mask, Wq, bq, Wk, bk, Wv, bv, trace=False):
    """Build + run; returns (output, BassKernelResults)."""
    in_maps = _prep_inputs(x, mask, Wq, bq, Wk, bk, Wv, bv)
    nc = _build_nc()
    res = run_bass_kernel_spmd(nc, in_maps, list(range(N_CORES)), trace=trace)
    out = np.empty((4, S, D), dtype=np.float32)
    for c in range(N_CORES):
        b, h = divmod(c, 2)
        out[b, h * QL:(h + 1) * QL, :] = res.results[c]["out"]
    return out, res


def kernel(x, mask, Wq, bq, Wk, bk, Wv, bv):
    out, _ = run(x, mask, Wq, bq, Wk, bk, Wv, bv)
    return out
